# revision 30
# baseline (speedup 1.0000x reference)
"""Trainium2 Bass kernel: causal self-attention with RoPE.

Sharding: tensor-parallel on the head axis. 16 heads over 8 cores = 2 heads
per core. Each core computes q/k/v projections for its 2 heads (from the
full, replicated input), runs causal attention for those heads over both
batch elements, and applies its slice of the output projection, producing a
partial [B*S, E] output (fp16). The host sums the 8 partials.

v2 design notes (vs the original baseline):
  - fp16 everywhere (was bf16): same PE speed, 8x less quantization error,
    half the output-DMA bytes. exp uses bias=-4 so e^(max logit) stays in
    fp16 range.
  - Softmax normalizer via DVE reciprocal_approx_fast (was Ln+Exp on the
    scalar engine, which thrashed ACT table loads ~2.7us per swap).
  - RoPE is fused into the q/k projection extraction: cos-multiply reads
    the projection result straight out of PSUM (DVE), the shuffle also
    reads PSUM, and the sin-multiply + add run on GPSIMD to keep DVE free.
  - v-projection accumulates 4 s-tiles into one PSUM tile and extracts
    with a single strided copy.
  - out-projection extraction alternates DVE/ACT per s-tile; fp16 ob tile
    per gst with a single 256KB DMA.
  - PE warm-up runs on a memset tile from t=0 (no DMA dependency) and a
    tiny dummy exp preloads the ACT exp table early.
  - Emission keeps the PE dense through both batches' attention with a
    hand-balanced fill schedule (projections of b1 inside b0's attention,
    out-projections threaded through both phases).
"""

import functools

import numpy as np
import ml_dtypes

import concourse.bass as bass
import concourse.mybir as mybir
import concourse.tile as tile
from concourse import bacc
from concourse.bass_utils import run_bass_kernel_spmd

F32 = mybir.dt.float32
F16 = mybir.dt.float16
BF16 = mybir.dt.bfloat16
NPF16 = np.float16
NPBF16 = ml_dtypes.bfloat16

E = 1024
HD = 64
N_CORES = 8
ROPE_BASE = 10000.0
EXP_BIAS = -4.0


def _build(seq: int, nb: int) -> bacc.Bacc:
    TS = nb * seq                 # total sequence columns (batches concatenated)
    QC = min(512, seq)            # q-chunk width for attention
    NQC = seq // QC               # q-chunks per batch
    NKTB = seq // 128             # k-tiles per batch
    NET = E // 128                # contraction tiles = 8
    PCB = min(512, seq)           # per-batch projection s-chunk
    NPCB = seq // PCB
    NSTB = seq // 128             # s-tiles per batch
    SPC = QC // 128               # s-tiles per q-chunk

    nc = bacc.Bacc(
        "TRN2",
        target_bir_lowering=False,
        debug=False,
        enable_asserts=False,
        num_devices=N_CORES,
    )

    xT_d = nc.dram_tensor("xT", [E, TS], BF16, kind="ExternalInput").ap()
    wq_d = nc.dram_tensor("wqT", [E, 128], BF16, kind="ExternalInput").ap()
    wk_d = nc.dram_tensor("wkT", [E, 128], BF16, kind="ExternalInput").ap()
    wv_d = nc.dram_tensor("wvT", [E, 128], BF16, kind="ExternalInput").ap()
    wo_d = nc.dram_tensor("woT", [128, E], BF16, kind="ExternalInput").ap()
    cos_d = nc.dram_tensor("cosT", [128, seq], BF16, kind="ExternalInput").ap()
    sin_d = nc.dram_tensor("sinT", [128, seq], BF16, kind="ExternalInput").ap()
    tri_d = nc.dram_tensor("tri", [128, 128], BF16, kind="ExternalInput").ap()
    out_d = nc.dram_tensor("out_p", [TS, E], F16, kind="ExternalOutput").ap()
    recd = [nc.dram_tensor(f"rec_scratch{b}", [2 * NQC, QC], F32).ap()
            for b in range(nb)]

    with tile.TileContext(nc) as tc:
        with (
            tc.tile_pool(name="persist", bufs=1) as persist,
            tc.tile_pool(name="pt", bufs=6) as ptp,
            tc.tile_pool(name="ob", bufs=4) as obp,
            tc.tile_pool(name="rec", bufs=4) as recp,
            tc.tile_pool(name="ps_big", bufs=2, space="PSUM") as psb,
            tc.tile_pool(name="ps_ov", bufs=2, space="PSUM") as psov,
            tc.tile_pool(name="ps_e", bufs=2, space="PSUM") as pse,
        ):
            def T(shape, dtype, name):
                return persist.tile(shape, dtype, name=name, tag=name)

            # ---- weights / tables (DMA order matters: earliest-needed first)
            wq_s = T([128, NET, 128], BF16, "wq_s")
            wk_s = T([128, NET, 128], BF16, "wk_s")
            wv_s = T([128, NET, 128], BF16, "wv_s")
            wo_s = T([128, E], BF16, "wo_s")
            cos_s = T([128, seq], BF16, "cos_s")
            sin_s = T([128, seq], BF16, "sin_s")
            tri_s = T([128, 128], BF16, "tri_s")

            nc.sync.dma_start(out=wq_s, in_=wq_d.rearrange("(t p) d -> p t d", p=128))

            # ---- PE warm-up from t=0 on a memset tile (no DMA dependency).
            # HAM needs ~3.4us of sustained activity to unthrottle 1.2->2.4GHz.
            warm_src = T([128, 128], BF16, "warm_src")
            nc.vector.memset(warm_src, 0.125)
            ebias = T([128, 1], F32, "ebias")
            nc.gpsimd.memset(ebias, EXP_BIAS)
            ones16 = T([1, 64], BF16, "ones16")
            nc.gpsimd.memset(ones16, 1.0)
            wu = psb.tile([128, 2, QC], F32, tag="psb", name="warmup")
            for _ in range(48):
                nc.tensor.matmul(wu[:, 0, 0:128], lhsT=warm_src,
                                 rhs=warm_src, start=True, stop=True)
            # preload the exp ACT table while DMAs stream
            ptw = ptp.tile([128, 2, QC], BF16, tag="pt", name="ptwarm")
            nc.scalar.activation(ptw[:, 0, 0:64], wu[:, 0, 0:64],
                                 mybir.ActivationFunctionType.Exp, bias=ebias)

            # ---- resident input: one tile + one DMA per (batch, s-chunk)
            xts = {}

            def emit_xt(b, pc):
                xt = T([128, NET, PCB], BF16, f"xt{b}_{pc}")
                nc.sync.dma_start(
                    out=xt,
                    in_=xT_d[:, b * seq + pc * PCB:
                            b * seq + (pc + 1) * PCB].rearrange(
                                "(t p) c -> p t c", p=128))
                xts[(b, pc)] = xt

            emit_xt(0, 0)
            nc.sync.dma_start(out=wk_s, in_=wk_d.rearrange("(t p) d -> p t d", p=128))
            nc.sync.dma_start(out=wv_s, in_=wv_d.rearrange("(t p) d -> p t d", p=128))
            nc.sync.dma_start(out=cos_s, in_=cos_d)
            nc.sync.dma_start(out=sin_s, in_=sin_d)
            nc.sync.dma_start(out=tri_s, in_=tri_d)
            emit_xt(0, 1)
            emit_xt(0, 2)
            emit_xt(0, 3)
            nc.sync.dma_start(out=wo_s, in_=wo_d)
            for pc in range(NPCB):
                emit_xt(1, pc)

            qT = T([128, TS], BF16, "qT")
            kT = T([128, TS], BF16, "kT")
            vo = T([128, nb * NKTB, 130], BF16, "vo")   # [vA|1|vB|1] per k-tile
            oT = T([128, TS], BF16, "oT")
            nc.gpsimd.memset(vo, 1.0)

            # ---------- emission helpers ----------
            def proj_qk_pieces(b, pc):
                """q+k projection for one 512-col chunk, RoPE fused into the
                PSUM extraction. 8 pieces of 2 matmuls each."""
                cols = slice(b * seq + pc * PCB, b * seq + (pc + 1) * PCB)
                tcols = slice(pc * PCB, (pc + 1) * PCB)
                pieces = []
                state = {}
                for wi, (w_s, dst) in enumerate(((wq_s, qT), (wk_s, kT))):
                    for e0 in range(0, NET, 2):
                        def piece(wi=wi, w_s=w_s, dst=dst, e0=e0):
                            if e0 == 0:
                                state[wi] = pse.tile(
                                    [128, PCB], F32, tag="pse",
                                    name=f"qk{b}_{pc}_{wi}")
                            ps = state[wi]
                            for et in (e0, e0 + 1):
                                nc.tensor.matmul(
                                    ps, lhsT=w_s[:, et, :],
                                    rhs=xts[(b, pc)][:, et, :],
                                    start=(et == 0), stop=(et == NET - 1),
                                )
                            if e0 + 2 == NET:
                                # fused extract + RoPE:
                                #   dst = ps*cos + shuffle(ps)*sin_signed
                                sh = recp.tile([128, PCB], F32, tag="ropesh",
                                               name=f"sh{wi}{b}_{pc}")
                                sh2 = recp.tile([128, PCB], BF16, tag="ropesh2",
                                                name=f"sg{wi}{b}_{pc}")
                                nc.vector.stream_shuffle(
                                    sh, ps, [i ^ 1 for i in range(32)])
                                nc.vector.tensor_mul(
                                    dst[:, cols], ps, cos_s[:, tcols])
                                nc.vector.tensor_mul(sh2, sh, sin_s[:, tcols])
                                nc.gpsimd.tensor_add(
                                    dst[:, cols], dst[:, cols], sh2)
                        pieces.append(piece)
                return pieces

            def proj_v_pieces2(b, g2):
                """v projection for 2 s-tiles into one PSUM tile; single
                strided extraction into vo. 4 pieces of 4 matmuls."""
                state = {}
                pieces = []
                for sti in range(2):
                    for e0 in (0, 4):
                        def piece(sti=sti, e0=e0):
                            if sti == 0 and e0 == 0:
                                state[0] = pse.tile([128, 2, 128], F32,
                                                    tag="pse",
                                                    name=f"v{b}_{g2}")
                            ps = state[0]
                            st = 2 * g2 + sti
                            pc, c0 = st // 4, (st % 4) * 128
                            for et in range(e0, e0 + 4):
                                nc.tensor.matmul(
                                    ps[:, sti, :],
                                    lhsT=xts[(b, pc)][:, et, c0:c0 + 128],
                                    rhs=wv_s[:, et, :],
                                    start=(et == 0), stop=(et == NET - 1),
                                )
                            if sti == 1 and e0 == 4:
                                base = vo[:, b * NKTB + 2 * g2, :]
                                dst = bass.AP(
                                    tensor=base.tensor, offset=base.offset,
                                    ap=[list(base.ap[0]), [130, 2],
                                        [65, 2], [1, 64]])
                                nc.vector.tensor_copy(
                                    out=dst,
                                    in_=ps.rearrange("p s (h d) -> p s h d",
                                                     d=64))
                        pieces.append(piece)
                return pieces

            pts_cache = {}

            def d1_kj(b, c, kj):
                qbase = c * QC
                gq0 = b * seq + qbase
                o = kj * 128 - qbase
                ro = max(o, 0)
                nj = QC - ro
                kc = b * seq + kj * 128
                ps = psb.tile([128, 2, QC], F32, tag="psb",
                              name=f"ss{b}_{c}_{kj}")
                for h in range(2):
                    rows = slice(h * 64, h * 64 + 64)
                    nc.tensor.matmul(
                        ps[:, h, 0:nj],
                        lhsT=kT[rows, kc:kc + 128],
                        rhs=qT[rows, gq0 + ro:gq0 + QC],
                        start=True, stop=True,
                        tile_position=(h * 64, 0),
                    )
                pt = ptp.tile([128, 2, QC], BF16, tag="pt",
                              name=f"pt{b}_{c}_{kj}")
                nc.scalar.activation(
                    pt[:, :, 0:nj], ps[:, :, 0:nj],
                    mybir.ActivationFunctionType.Exp, bias=ebias,
                )
                if o >= 0:
                    tri_b = bass.AP(
                        tensor=tri_s.tensor, offset=tri_s.offset,
                        ap=[list(tri_s.ap[0]), [0, 2], list(tri_s.ap[1])],
                    )
                    nc.gpsimd.tensor_mul(
                        pt[:, :, 0:128], pt[:, :, 0:128], tri_b)
                return pt, ro, nj

            def d2_kj(b, c, kj, ops_, nkt):
                pt, ro, nj = pts_cache[(b, c, kj)]
                for h in range(2):
                    nc.tensor.matmul(
                        ops_[h][:, ro:QC],
                        lhsT=vo[:, b * NKTB + kj, h * 65:h * 65 + 65],
                        rhs=pt[:, h, 0:nj],
                        start=(kj == 0), stop=(kj == nkt - 1),
                    )

            def attn_chunk(b, c, fills):
                qbase = c * QC
                nkt = (qbase + QC) // 128
                ops_ = [psov.tile([65, QC], F32, tag="psov", name=f"o{b}_{c}_{h}")
                        for h in range(2)]
                fq = list(fills)
                for kj in range(nkt):
                    pt, ro, nj = d1_kj(b, c, kj)
                    if kj >= 2:
                        d2_kj(b, c, kj - 2, ops_, nkt)
                    for _ in range(2):
                        if fq:
                            fq.pop(0)()
                    pts_cache[(b, c, kj)] = (pt, ro, nj)
                for kj in range(max(nkt - 2, 0), nkt):
                    d2_kj(b, c, kj, ops_, nkt)
                for p in fq:
                    p()
                return ops_

            def d3_norm(b, c, ops_, via_pe=False):
                gq0 = b * seq + c * QC
                rec = [recp.tile([1, QC], F32, tag="rec", name=f"rc{b}_{c}_{h}")
                       for h in range(2)]
                if via_pe:
                    # tail chunk: on-chip broadcast via PE; extraction runs on
                    # ACT (exp stream is done by now) so DVE only does the
                    # reciprocal and the final multiply
                    rb_ps = pse.tile([128, QC], F32, tag="pse",
                                     name=f"rbp{b}_{c}")
                    rc16 = [recp.tile([1, QC], BF16, tag="rec16",
                                      name=f"r6{b}_{c}_{h}") for h in range(2)]
                    for h in range(2):
                        op = ops_[h]
                        dcc = recp.tile([1, QC], F32, tag="dch",
                                        name=f"dc{b}_{c}_{h}")
                        nc.scalar.copy(out=dcc, in_=op[64:65, 0:QC])
                        nc.vector.reciprocal_approx_fast(rec[h], dcc)
                        nc.scalar.copy(
                            out=oT[h * 64:h * 64 + 64, gq0:gq0 + QC],
                            in_=op[0:64, 0:QC])
                        nc.vector.tensor_copy(out=rc16[h], in_=rec[h])
                        nc.tensor.matmul(
                            rb_ps[h * 64:h * 64 + 64, :],
                            lhsT=ones16, rhs=rc16[h],
                            start=True, stop=True,
                            tile_position=(0, h * 64),
                        )
                    nc.vector.tensor_mul(
                        oT[:, gq0:gq0 + QC], oT[:, gq0:gq0 + QC], rb_ps)
                    return
                for h in range(2):
                    op = ops_[h]
                    nc.vector.tensor_copy(
                        out=oT[h * 64:h * 64 + 64, gq0:gq0 + QC],
                        in_=op[0:64, 0:QC])
                    dcc = recp.tile([1, QC], F32, tag="dch",
                                    name=f"dc{b}_{c}_{h}")
                    nc.vector.tensor_copy(out=dcc, in_=op[64:65, 0:QC])
                    nc.vector.reciprocal_approx_fast(rec[h], dcc)
                for h in range(2):
                    nc.sync.dma_start(
                        out=recd[b][2 * c + h:2 * c + h + 1, :],
                        in_=rec[h])
                rb = recp.tile([128, QC], F32, tag="rb", name=f"rb{b}_{c}")
                for h in range(2):
                    row = recd[b][2 * c + h:2 * c + h + 1, :]
                    bcast = bass.AP(tensor=row.tensor, offset=row.offset,
                                    ap=[[0, 64], [1, QC]])
                    nc.sync.dma_start(out=rb[h * 64:h * 64 + 64, :], in_=bcast)
                nc.vector.tensor_mul(
                    oT[:, gq0:gq0 + QC], oT[:, gq0:gq0 + QC], rb)

            def eproj_pieces(gst, mode):
                """out-projection for one 128-col s-tile: 2 matmuls, 2 PSUM
                extractions, one 256KB DMA. mode picks the extraction engine
                split: 'A' = 1:1 DVE/ACT, 'D' = 2:1 DVE-leaning."""
                state = {}

                def piece(ec, gst=gst):
                    if ec == 0:
                        state["ob"] = obp.tile([128, 2, 512], F16, tag="ob",
                                               name=f"ob{gst}")
                    ob = state["ob"]
                    ps = pse.tile([128, 512], F32, tag="pse",
                                  name=f"op{gst}_{ec}")
                    nc.tensor.matmul(
                        ps,
                        lhsT=oT[:, gst * 128:(gst + 1) * 128],
                        rhs=wo_s[:, ec * 512:(ec + 1) * 512],
                        start=True, stop=True,
                    )
                    idx = gst * 2 + ec
                    on_act = (idx % 2 == 1) if mode == "A" else (idx % 3 == 2)
                    if on_act:
                        nc.scalar.copy(out=ob[:, ec, :], in_=ps)
                    else:
                        nc.vector.tensor_copy(out=ob[:, ec, :], in_=ps)
                    if ec == 1:
                        nc.sync.dma_start(
                            out=out_d[gst * 128:(gst + 1) * 128, :],
                            in_=ob.rearrange("p a b -> p (a b)"),
                        )
                return [lambda ec=ec: piece(ec) for ec in (0, 1)]

            def qk(b, pc):
                return proj_qk_pieces(b, pc)

            def v2g(b, g):
                """v pieces for s-tiles [2g, 2g+2)."""
                return proj_v_pieces2(b, g)

            def vc(b, c):
                """v pieces for the 4 s-tiles of chunk c (two groups)."""
                return proj_v_pieces2(b, 2 * c) + proj_v_pieces2(b, 2 * c + 1)

            def ep(b, c, mode="D"):
                """eproj pieces for the 4 s-tiles of chunk c of batch b."""
                out = []
                for st in range(c * SPC, (c + 1) * SPC):
                    out.extend(eproj_pieces(b * NSTB + st, mode))
                return out

            # ---------- emission ----------
            # prefix: all of b0's q/k projections (DMA-paced, keeps the PE
            # continuously busy from t~4us so HAM warms once and stays warm)
            for pc in range(NPCB):
                for p in qk(0, pc):
                    p()

            # phase A: b0 attention, largest chunk first. Chunk c has
            # 8*(c+1) fill slots. b0's v-projection streams as c3's fills
            # (group g covers k-tiles 2g..2g+1, needed at step 2g+2 --
            # popped by step ~g+1, always ahead). b1's projections fill the
            # rest; b0c3's out-projection lands in the last (smallest) chunk.
            fills_A = {
                3: [v2g(0, g) for g in range(8)],
                2: [qk(1, 0), qk(1, 1), v2g(1, 0), v2g(1, 1)],
                1: [qk(1, 2), qk(1, 3)],
                0: [v2g(1, 2), v2g(1, 3)],
            }
            for c in (3, 2, 1, 0):
                fills = [p for grp in fills_A[c] for p in grp]
                if c == 0:
                    fills += ep(0, 3, "A")
                ops_ = attn_chunk(0, c, fills)
                d3_norm(0, c, ops_)

            # phase B: b1 attention, largest first; remaining v groups are
            # c3's fills, out-projections thread through every chunk.
            fills_B = {
                3: [v2g(1, g) for g in range(4, 8)],
                2: [],
                1: [],
                0: [],
            }
            for c in (3, 2, 1, 0):
                fills = [p for grp in fills_B[c] for p in grp]
                if c == 3:
                    fills += ep(0, 2, "D")
                elif c == 2:
                    fills += ep(0, 1, "D") + ep(0, 0, "D") + ep(1, 3, "D")
                elif c == 1:
                    fills += ep(1, 2, "D")
                else:
                    fills += ep(1, 1, "D")
                ops_ = attn_chunk(1, c, fills)
                d3_norm(1, c, ops_, via_pe=(c == 0))
            for p in ep(1, 0, "A"):
                p()

    nc.compile()
    return nc


@functools.lru_cache(maxsize=2)
def _built(seq: int, nb: int) -> bacc.Bacc:
    return _build(seq, nb)


def _host_tables(seq: int):
    inv = 1.0 / (ROPE_BASE ** (np.arange(0, HD, 2, dtype=np.float32) / HD))
    f = np.outer(np.arange(seq, dtype=np.float32), inv)
    emb = np.concatenate([f, f], axis=-1)        # [S, 64] (concat layout)
    cos = np.cos(emb).T.astype(np.float32)       # [64, S]
    sin = np.sin(emb).T.astype(np.float32)
    sgn = np.where(np.arange(HD) % 2 == 0, -1.0, 1.0).astype(np.float32)
    sin_signed = sin * sgn[:, None]
    cosT = np.concatenate([cos, cos], axis=0).astype(NPBF16)       # [128, S]
    sinT = np.concatenate([sin_signed, sin_signed], axis=0).astype(NPBF16)
    return cosT, sinT


def make_in_maps(x, Wq, Wk, Wv, Wo):
    x = np.asarray(x, dtype=np.float32)
    B, S, E_ = x.shape
    assert E_ == E
    xT = np.ascontiguousarray(x.reshape(B * S, E_).T).astype(NPBF16)  # [E, B*S]
    cosT, sinT = _host_tables(S)
    i_idx = np.arange(128)
    tri = (i_idx[None, :] >= i_idx[:, None]).astype(NPBF16)  # keep j >= i
    scale = np.float32(HD ** -0.5)
    in_maps = []
    for core in range(N_CORES):
        cols = slice(core * 128, core * 128 + 128)   # heads 2c, 2c+1 dims
        wqT = np.ascontiguousarray((np.asarray(Wq)[cols, :] * scale).T).astype(NPBF16)
        wkT = np.ascontiguousarray(np.asarray(Wk)[cols, :].T).astype(NPBF16)
        wvT = np.ascontiguousarray(np.asarray(Wv)[cols, :].T).astype(NPBF16)
        woT = np.ascontiguousarray(np.asarray(Wo)[:, cols].T).astype(NPBF16)
        in_maps.append(dict(
            xT=xT, wqT=wqT, wkT=wkT, wvT=wvT, woT=woT,
            cosT=cosT, sinT=sinT, tri=tri,
        ))
    return in_maps


def kernel(x, Wq, Wk, Wv, Wo):
    x = np.asarray(x, dtype=np.float32)
    B, S, E_ = x.shape
    nc = _built(S, B)
    in_maps = make_in_maps(x, Wq, Wk, Wv, Wo)
    res = run_bass_kernel_spmd(nc, in_maps, core_ids=list(range(N_CORES)))
    out = np.zeros((B * S, E_), np.float32)
    for r in res.results:
        out += r["out_p"]
    return out.reshape(B, S, E_)


# revision 32
# speedup vs baseline: 1.0603x; 1.0603x over previous
"""Trainium2 Bass kernel: causal self-attention with RoPE.

Sharding: tensor-parallel on the head axis. 16 heads over 8 cores = 2 heads
per core. Each core computes q/k/v projections for its 2 heads (from the
full, replicated input), runs causal attention for those heads over both
batch elements, and applies its slice of the output projection, producing a
partial [B*S, E] output (fp16). The host sums the 8 partials.

v2 design notes (vs the original baseline):
  - fp16 everywhere (was bf16): same PE speed, 8x less quantization error,
    half the output-DMA bytes. exp uses bias=-4 so e^(max logit) stays in
    fp16 range.
  - Softmax normalizer via DVE reciprocal_approx_fast (was Ln+Exp on the
    scalar engine, which thrashed ACT table loads ~2.7us per swap).
  - RoPE is fused into the q/k projection extraction: cos-multiply reads
    the projection result straight out of PSUM (DVE), the shuffle also
    reads PSUM, and the sin-multiply + add run on GPSIMD to keep DVE free.
  - v-projection accumulates 4 s-tiles into one PSUM tile and extracts
    with a single strided copy.
  - out-projection extraction alternates DVE/ACT per s-tile; fp16 ob tile
    per gst with a single 256KB DMA.
  - PE warm-up runs on a memset tile from t=0 (no DMA dependency) and a
    tiny dummy exp preloads the ACT exp table early.
  - Emission keeps the PE dense through both batches' attention with a
    hand-balanced fill schedule (projections of b1 inside b0's attention,
    out-projections threaded through both phases).
"""

import functools

import numpy as np
import ml_dtypes

import concourse.bass as bass
import concourse.mybir as mybir
import concourse.tile as tile
from concourse import bacc
from concourse.bass_utils import run_bass_kernel_spmd

F32 = mybir.dt.float32
F16 = mybir.dt.float16
BF16 = mybir.dt.bfloat16
NPF16 = np.float16
NPBF16 = ml_dtypes.bfloat16

E = 1024
HD = 64
N_CORES = 8
ROPE_BASE = 10000.0
EXP_BIAS = -4.0


def _build(seq: int, nb: int) -> bacc.Bacc:
    TS = nb * seq                 # total sequence columns (batches concatenated)
    QC = min(512, seq)            # q-chunk width for attention
    NQC = seq // QC               # q-chunks per batch
    NKTB = seq // 128             # k-tiles per batch
    NET = E // 128                # contraction tiles = 8
    PCB = min(512, seq)           # per-batch projection s-chunk
    NPCB = seq // PCB
    NSTB = seq // 128             # s-tiles per batch
    SPC = QC // 128               # s-tiles per q-chunk

    nc = bacc.Bacc(
        "TRN2",
        target_bir_lowering=False,
        debug=False,
        enable_asserts=False,
        num_devices=N_CORES,
    )

    xT_d = nc.dram_tensor("xT", [E, TS], BF16, kind="ExternalInput").ap()
    wq_d = nc.dram_tensor("wqT", [E, 128], BF16, kind="ExternalInput").ap()
    wk_d = nc.dram_tensor("wkT", [E, 128], BF16, kind="ExternalInput").ap()
    wv_d = nc.dram_tensor("wvT", [E, 128], BF16, kind="ExternalInput").ap()
    wo_d = nc.dram_tensor("woT", [128, E], BF16, kind="ExternalInput").ap()
    cos_d = nc.dram_tensor("cosT", [128, seq], BF16, kind="ExternalInput").ap()
    sin_d = nc.dram_tensor("sinT", [128, seq], BF16, kind="ExternalInput").ap()
    tri_d = nc.dram_tensor("tri", [128, 128], BF16, kind="ExternalInput").ap()
    out_d = nc.dram_tensor("out_p", [TS, E], F16, kind="ExternalOutput").ap()
    recd = [nc.dram_tensor(f"rec_scratch{b}", [2 * NQC, QC], F32).ap()
            for b in range(nb)]

    with tile.TileContext(nc) as tc:
        with (
            tc.tile_pool(name="persist", bufs=1) as persist,
            tc.tile_pool(name="pt", bufs=6) as ptp,
            tc.tile_pool(name="ob", bufs=4) as obp,
            tc.tile_pool(name="rec", bufs=4) as recp,
            tc.tile_pool(name="ps_big", bufs=2, space="PSUM") as psb,
            tc.tile_pool(name="ps_ov", bufs=2, space="PSUM") as psov,
            tc.tile_pool(name="ps_e", bufs=2, space="PSUM") as pse,
        ):
            def T(shape, dtype, name):
                return persist.tile(shape, dtype, name=name, tag=name)

            # ---- weights / tables (DMA order matters: earliest-needed first)
            wq_s = T([128, NET, 128], BF16, "wq_s")
            wk_s = T([128, NET, 128], BF16, "wk_s")
            wv_s = T([128, NET, 128], BF16, "wv_s")
            wo_s = T([128, E], BF16, "wo_s")
            cos_s = T([128, seq], BF16, "cos_s")
            sin_s = T([128, seq], BF16, "sin_s")
            tri_s = T([128, 128], BF16, "tri_s")

            nc.sync.dma_start(out=wq_s, in_=wq_d.rearrange("(t p) d -> p t d", p=128))

            # ---- PE warm-up from t=0 on a memset tile (no DMA dependency).
            # HAM needs ~3.4us of sustained activity to unthrottle 1.2->2.4GHz.
            warm_src = T([128, 128], BF16, "warm_src")
            nc.vector.memset(warm_src, 0.125)
            ebias = T([128, 1], F32, "ebias")
            nc.gpsimd.memset(ebias, EXP_BIAS)
            ones16 = T([1, 64], BF16, "ones16")
            nc.gpsimd.memset(ones16, 1.0)
            wu = psb.tile([128, 2, QC], F32, tag="psb", name="warmup")
            for _ in range(48):
                nc.tensor.matmul(wu[:, 0, 0:128], lhsT=warm_src,
                                 rhs=warm_src, start=True, stop=True)
            # preload the exp ACT table while DMAs stream
            ptw = ptp.tile([128, 2, QC], BF16, tag="pt", name="ptwarm")
            nc.scalar.activation(ptw[:, 0, 0:64], wu[:, 0, 0:64],
                                 mybir.ActivationFunctionType.Exp, bias=ebias)

            # ---- resident input: one tile + one DMA per (batch, s-chunk)
            xts = {}

            def emit_xt(b, pc):
                xt = T([128, NET, PCB], BF16, f"xt{b}_{pc}")
                nc.sync.dma_start(
                    out=xt,
                    in_=xT_d[:, b * seq + pc * PCB:
                            b * seq + (pc + 1) * PCB].rearrange(
                                "(t p) c -> p t c", p=128))
                xts[(b, pc)] = xt

            emit_xt(0, 0)
            nc.sync.dma_start(out=wk_s, in_=wk_d.rearrange("(t p) d -> p t d", p=128))
            nc.sync.dma_start(out=wv_s, in_=wv_d.rearrange("(t p) d -> p t d", p=128))
            nc.sync.dma_start(out=cos_s, in_=cos_d)
            nc.sync.dma_start(out=sin_s, in_=sin_d)
            nc.sync.dma_start(out=tri_s, in_=tri_d)
            emit_xt(0, 1)
            emit_xt(0, 2)
            emit_xt(0, 3)
            nc.sync.dma_start(out=wo_s, in_=wo_d)
            for pc in range(NPCB):
                emit_xt(1, pc)

            qT = T([128, TS], BF16, "qT")
            kT = T([128, TS], BF16, "kT")
            vo = T([128, nb * NKTB, 130], BF16, "vo")   # [vA|1|vB|1] per k-tile
            oT = T([128, TS], BF16, "oT")
            nc.gpsimd.memset(vo, 1.0)

            # ---------- emission helpers ----------
            def proj_qk_pieces(b, pc):
                """q+k projection for one 512-col chunk, RoPE fused into the
                PSUM extraction. 8 pieces of 2 matmuls each."""
                cols = slice(b * seq + pc * PCB, b * seq + (pc + 1) * PCB)
                tcols = slice(pc * PCB, (pc + 1) * PCB)
                pieces = []
                state = {}
                for wi, (w_s, dst) in enumerate(((wq_s, qT), (wk_s, kT))):
                    for e0 in range(0, NET, 2):
                        def piece(wi=wi, w_s=w_s, dst=dst, e0=e0):
                            if e0 == 0:
                                state[wi] = pse.tile(
                                    [128, PCB], F32, tag="pse",
                                    name=f"qk{b}_{pc}_{wi}")
                            ps = state[wi]
                            for et in (e0, e0 + 1):
                                nc.tensor.matmul(
                                    ps, lhsT=w_s[:, et, :],
                                    rhs=xts[(b, pc)][:, et, :],
                                    start=(et == 0), stop=(et == NET - 1),
                                )
                            if e0 + 2 == NET:
                                # extract to SBUF first (frees the PSUM bank
                                # after one fast read), then RoPE runs
                                # SBUF-side where bf16 TT ops hit 2x mode:
                                #   dst = raw*cos + shuffle(raw)*sin_signed
                                raw = recp.tile([128, PCB], BF16, tag="roperaw",
                                                name=f"rw{wi}{b}_{pc}")
                                sh = recp.tile([128, PCB], BF16, tag="ropesh",
                                               name=f"sh{wi}{b}_{pc}")
                                nc.vector.tensor_copy(out=raw, in_=ps)
                                nc.vector.stream_shuffle(
                                    sh, raw, [i ^ 1 for i in range(32)])
                                nc.vector.tensor_mul(
                                    dst[:, cols], raw, cos_s[:, tcols])
                                nc.vector.tensor_mul(sh, sh, sin_s[:, tcols])
                                nc.gpsimd.tensor_add(
                                    dst[:, cols], dst[:, cols], sh)
                        pieces.append(piece)
                return pieces

            def proj_v_pieces2(b, g2):
                """v projection for 2 s-tiles into one PSUM tile; single
                strided extraction into vo. 4 pieces of 4 matmuls."""
                state = {}
                pieces = []
                for sti in range(2):
                    for e0 in (0, 4):
                        def piece(sti=sti, e0=e0):
                            if sti == 0 and e0 == 0:
                                state[0] = pse.tile([128, 2, 128], F32,
                                                    tag="pse",
                                                    name=f"v{b}_{g2}")
                            ps = state[0]
                            st = 2 * g2 + sti
                            pc, c0 = st // 4, (st % 4) * 128
                            for et in range(e0, e0 + 4):
                                nc.tensor.matmul(
                                    ps[:, sti, :],
                                    lhsT=xts[(b, pc)][:, et, c0:c0 + 128],
                                    rhs=wv_s[:, et, :],
                                    start=(et == 0), stop=(et == NET - 1),
                                )
                            if sti == 1 and e0 == 4:
                                base = vo[:, b * NKTB + 2 * g2, :]
                                dst = bass.AP(
                                    tensor=base.tensor, offset=base.offset,
                                    ap=[list(base.ap[0]), [130, 2],
                                        [65, 2], [1, 64]])
                                nc.vector.tensor_copy(
                                    out=dst,
                                    in_=ps.rearrange("p s (h d) -> p s h d",
                                                     d=64))
                        pieces.append(piece)
                return pieces

            pts_cache = {}

            def d1_kj(b, c, kj):
                qbase = c * QC
                gq0 = b * seq + qbase
                o = kj * 128 - qbase
                ro = max(o, 0)
                nj = QC - ro
                kc = b * seq + kj * 128
                ps = psb.tile([128, 2, QC], F32, tag="psb",
                              name=f"ss{b}_{c}_{kj}")
                for h in range(2):
                    rows = slice(h * 64, h * 64 + 64)
                    nc.tensor.matmul(
                        ps[:, h, 0:nj],
                        lhsT=kT[rows, kc:kc + 128],
                        rhs=qT[rows, gq0 + ro:gq0 + QC],
                        start=True, stop=True,
                        tile_position=(h * 64, 0),
                    )
                pt = ptp.tile([128, 2, QC], BF16, tag="pt",
                              name=f"pt{b}_{c}_{kj}")
                nc.scalar.activation(
                    pt[:, :, 0:nj], ps[:, :, 0:nj],
                    mybir.ActivationFunctionType.Exp, bias=ebias,
                )
                if o >= 0:
                    tri_b = bass.AP(
                        tensor=tri_s.tensor, offset=tri_s.offset,
                        ap=[list(tri_s.ap[0]), [0, 2], list(tri_s.ap[1])],
                    )
                    nc.gpsimd.tensor_mul(
                        pt[:, :, 0:128], pt[:, :, 0:128], tri_b)
                return pt, ro, nj

            def d2_kj(b, c, kj, ops_, nkt):
                pt, ro, nj = pts_cache[(b, c, kj)]
                for h in range(2):
                    nc.tensor.matmul(
                        ops_[h][:, ro:QC],
                        lhsT=vo[:, b * NKTB + kj, h * 65:h * 65 + 65],
                        rhs=pt[:, h, 0:nj],
                        start=(kj == 0), stop=(kj == nkt - 1),
                    )

            def attn_chunk(b, c, fills):
                qbase = c * QC
                nkt = (qbase + QC) // 128
                ops_ = [psov.tile([65, QC], F32, tag="psov", name=f"o{b}_{c}_{h}")
                        for h in range(2)]
                fq = list(fills)
                for kj in range(nkt):
                    pt, ro, nj = d1_kj(b, c, kj)
                    if kj >= 2:
                        d2_kj(b, c, kj - 2, ops_, nkt)
                    for _ in range(2):
                        if fq:
                            fq.pop(0)()
                    pts_cache[(b, c, kj)] = (pt, ro, nj)
                for kj in range(max(nkt - 2, 0), nkt):
                    d2_kj(b, c, kj, ops_, nkt)
                for p in fq:
                    p()
                return ops_

            def d3_norm(b, c, ops_, via_pe=False):
                gq0 = b * seq + c * QC
                rec = [recp.tile([1, QC], F32, tag="rec", name=f"rc{b}_{c}_{h}")
                       for h in range(2)]
                if via_pe:
                    # tail chunk: on-chip broadcast via PE; extraction runs on
                    # ACT (exp stream is done by now) so DVE only does the
                    # reciprocal and the final multiply
                    rb_ps = pse.tile([128, QC], F32, tag="pse",
                                     name=f"rbp{b}_{c}")
                    rc16 = [recp.tile([1, QC], BF16, tag="rec16",
                                      name=f"r6{b}_{c}_{h}") for h in range(2)]
                    for h in range(2):
                        op = ops_[h]
                        dcc = recp.tile([1, QC], F32, tag="dch",
                                        name=f"dc{b}_{c}_{h}")
                        nc.scalar.copy(out=dcc, in_=op[64:65, 0:QC])
                        nc.vector.reciprocal_approx_fast(rec[h], dcc)
                        nc.scalar.copy(
                            out=oT[h * 64:h * 64 + 64, gq0:gq0 + QC],
                            in_=op[0:64, 0:QC])
                        nc.vector.tensor_copy(out=rc16[h], in_=rec[h])
                        nc.tensor.matmul(
                            rb_ps[h * 64:h * 64 + 64, :],
                            lhsT=ones16, rhs=rc16[h],
                            start=True, stop=True,
                            tile_position=(0, h * 64),
                        )
                    nc.vector.tensor_mul(
                        oT[:, gq0:gq0 + QC], oT[:, gq0:gq0 + QC], rb_ps)
                    return
                for h in range(2):
                    op = ops_[h]
                    nc.vector.tensor_copy(
                        out=oT[h * 64:h * 64 + 64, gq0:gq0 + QC],
                        in_=op[0:64, 0:QC])
                    dcc = recp.tile([1, QC], F32, tag="dch",
                                    name=f"dc{b}_{c}_{h}")
                    nc.vector.tensor_copy(out=dcc, in_=op[64:65, 0:QC])
                    nc.vector.reciprocal_approx_fast(rec[h], dcc)
                for h in range(2):
                    nc.sync.dma_start(
                        out=recd[b][2 * c + h:2 * c + h + 1, :],
                        in_=rec[h])
                rb = recp.tile([128, QC], F32, tag="rb", name=f"rb{b}_{c}")
                for h in range(2):
                    row = recd[b][2 * c + h:2 * c + h + 1, :]
                    bcast = bass.AP(tensor=row.tensor, offset=row.offset,
                                    ap=[[0, 64], [1, QC]])
                    nc.sync.dma_start(out=rb[h * 64:h * 64 + 64, :], in_=bcast)
                nc.vector.tensor_mul(
                    oT[:, gq0:gq0 + QC], oT[:, gq0:gq0 + QC], rb)

            def eproj_pieces(gst, mode):
                """out-projection for one 128-col s-tile: 2 matmuls, 2 PSUM
                extractions, one 256KB DMA. mode picks the extraction engine
                split: 'A' = 1:1 DVE/ACT, 'D' = 2:1 DVE-leaning."""
                state = {}

                def piece(ec, gst=gst):
                    if ec == 0:
                        state["ob"] = obp.tile([128, 2, 512], F16, tag="ob",
                                               name=f"ob{gst}")
                    ob = state["ob"]
                    ps = pse.tile([128, 512], F32, tag="pse",
                                  name=f"op{gst}_{ec}")
                    nc.tensor.matmul(
                        ps,
                        lhsT=oT[:, gst * 128:(gst + 1) * 128],
                        rhs=wo_s[:, ec * 512:(ec + 1) * 512],
                        start=True, stop=True,
                    )
                    idx = gst * 2 + ec
                    on_act = (idx % 2 == 1) if mode == "A" else (idx % 3 == 2)
                    if on_act:
                        nc.scalar.copy(out=ob[:, ec, :], in_=ps)
                    else:
                        nc.vector.tensor_copy(out=ob[:, ec, :], in_=ps)
                    if ec == 1:
                        nc.sync.dma_start(
                            out=out_d[gst * 128:(gst + 1) * 128, :],
                            in_=ob.rearrange("p a b -> p (a b)"),
                        )
                return [lambda ec=ec: piece(ec) for ec in (0, 1)]

            def qk(b, pc):
                return proj_qk_pieces(b, pc)

            def v2g(b, g):
                """v pieces for s-tiles [2g, 2g+2)."""
                return proj_v_pieces2(b, g)

            def vc(b, c):
                """v pieces for the 4 s-tiles of chunk c (two groups)."""
                return proj_v_pieces2(b, 2 * c) + proj_v_pieces2(b, 2 * c + 1)

            def ep(b, c, mode="D"):
                """eproj pieces for the 4 s-tiles of chunk c of batch b."""
                out = []
                for st in range(c * SPC, (c + 1) * SPC):
                    out.extend(eproj_pieces(b * NSTB + st, mode))
                return out

            # ---------- emission ----------
            # prefix: minimal b0 projections so attention c0 can start early
            for p in qk(0, 0):
                p()
            for p in vc(0, 0):
                p()

            # fill schedule; chunk c has 8*(c+1) fill slots
            fills_A = {
                0: qk(0, 1),
                1: vc(0, 1) + qk(0, 2),
                2: vc(0, 2) + qk(0, 3) + vc(0, 3),
                3: None,  # built after norm of c0..c2 exists
            }
            for c in range(NQC):
                if c == 3:
                    fills = (ep(0, 0, "A") + qk(1, 0) + vc(1, 0)
                             + ep(0, 1, "A"))
                else:
                    fills = fills_A[c]
                ops_ = attn_chunk(0, c, fills)
                d3_norm(0, c, ops_)

            fills_B = {
                0: qk(1, 1),
                1: vc(1, 1) + qk(1, 2),
                2: vc(1, 2) + qk(1, 3) + ep(0, 2, "D"),
                3: None,
            }
            for c in range(NQC):
                if c == 3:
                    fills = (vc(1, 3) + ep(0, 3, "D") + ep(1, 0, "D")
                             + ep(1, 1, "D") + ep(1, 2, "D"))
                else:
                    fills = fills_B[c]
                ops_ = attn_chunk(1, c, fills)
                d3_norm(1, c, ops_, via_pe=(c == 3))
            for p in ep(1, 3, "A"):
                p()

    nc.compile()
    return nc


@functools.lru_cache(maxsize=2)
def _built(seq: int, nb: int) -> bacc.Bacc:
    return _build(seq, nb)


def _host_tables(seq: int):
    inv = 1.0 / (ROPE_BASE ** (np.arange(0, HD, 2, dtype=np.float32) / HD))
    f = np.outer(np.arange(seq, dtype=np.float32), inv)
    emb = np.concatenate([f, f], axis=-1)        # [S, 64] (concat layout)
    cos = np.cos(emb).T.astype(np.float32)       # [64, S]
    sin = np.sin(emb).T.astype(np.float32)
    sgn = np.where(np.arange(HD) % 2 == 0, -1.0, 1.0).astype(np.float32)
    sin_signed = sin * sgn[:, None]
    cosT = np.concatenate([cos, cos], axis=0).astype(NPBF16)       # [128, S]
    sinT = np.concatenate([sin_signed, sin_signed], axis=0).astype(NPBF16)
    return cosT, sinT


def make_in_maps(x, Wq, Wk, Wv, Wo):
    x = np.asarray(x, dtype=np.float32)
    B, S, E_ = x.shape
    assert E_ == E
    xT = np.ascontiguousarray(x.reshape(B * S, E_).T).astype(NPBF16)  # [E, B*S]
    cosT, sinT = _host_tables(S)
    i_idx = np.arange(128)
    tri = (i_idx[None, :] >= i_idx[:, None]).astype(NPBF16)  # keep j >= i
    scale = np.float32(HD ** -0.5)
    in_maps = []
    for core in range(N_CORES):
        cols = slice(core * 128, core * 128 + 128)   # heads 2c, 2c+1 dims
        wqT = np.ascontiguousarray((np.asarray(Wq)[cols, :] * scale).T).astype(NPBF16)
        wkT = np.ascontiguousarray(np.asarray(Wk)[cols, :].T).astype(NPBF16)
        wvT = np.ascontiguousarray(np.asarray(Wv)[cols, :].T).astype(NPBF16)
        woT = np.ascontiguousarray(np.asarray(Wo)[:, cols].T).astype(NPBF16)
        in_maps.append(dict(
            xT=xT, wqT=wqT, wkT=wkT, wvT=wvT, woT=woT,
            cosT=cosT, sinT=sinT, tri=tri,
        ))
    return in_maps


def kernel(x, Wq, Wk, Wv, Wo):
    x = np.asarray(x, dtype=np.float32)
    B, S, E_ = x.shape
    nc = _built(S, B)
    in_maps = make_in_maps(x, Wq, Wk, Wv, Wo)
    res = run_bass_kernel_spmd(nc, in_maps, core_ids=list(range(N_CORES)))
    out = np.zeros((B * S, E_), np.float32)
    for r in res.results:
        out += r["out_p"]
    return out.reshape(B, S, E_)
